# revision 21
# baseline (speedup 1.0000x reference)
"""Multi-head attention (B=2, N=2048, C=1024, H=16) on 8 Trainium2 cores.

Sharding: core cid = (b, hg) with b = cid//4, hg = cid%4.  Data-parallel on
batch, 4-way tensor-parallel on heads (4 heads / 256 dims per core).  Each
core computes q/k/v projections for its head slice, full (masked-softmax)
attention for its 4 heads, and a partial output projection y^T = Wp_slice^T
-contracted over its 256 dims.  Host sums the 4 partials per batch and adds
the proj bias.

All matmul inputs are bf16 (PSUM accumulation stays fp32).  The mask is
pre-cast to bf16 on the host and DMA'd in [128, NCK] pieces, paced so the
pieces for n-chunk ncb arrive one block ahead of use.

The kernel is a single software-pipelined stream: the qkv projections are
decomposed into 1-bank passes and woven between the attention iterations
of the first two blocks, so the Scalar engine's exp throughput -- the
pacing resource -- is never left idle behind a separate projection phase.
The V projection is computed in head-pair halves: the hp=0 block only
needs V for heads 0-1, so the h23 half rides in the hp=1 block, halving
the first block's weave burden.  Block order is ncb-outer / hp-inner; the
output projection for chunk ncb is woven into the (hp=0, ncb+1) block,
clear of the previous block's normalize chain.

Attention layout (all matmuls contract along the SBUF partition dim):
  - qk^T [512, 2048]: m-tiles 0/1 = Q^T head-pairs, 2/3 = K^T head-pairs
  - scores S^T[m, n] = K^T_h(stationary, 64 rows) x Q^T_h; the two heads
    of a pair run concurrently in different PE row groups
  - P^T = exp(S^T * scale) * mask^T (bf16)
  - O^T_aug[65, n] via V_aug^T-contraction (ones column = denominator);
    both head-pair halves accumulate into one 2-bank PSUM pair so each
    block needs a single reciprocal+broadcast normalize chain.
"""

import os
import sys
import types
from contextlib import ExitStack

import numpy as np
import ml_dtypes

import concourse.bass as bass
import concourse.mybir as mybir
import concourse.tile as tile
from concourse import bacc
from concourse.bass_utils import run_bass_kernel_spmd
from concourse.tile import add_dep_helper

# ---------------------------------------------------------------- constants
N = 2048          # sequence length
C = 1024          # model dim
NH = 4            # heads per core
HD = 64           # head dim
DQK = 2 * NH * HD # 512: q rows then k rows in qk^T
DV = NH * HD      # 256
NCK = 512         # n-chunk size
NCH = N // NCK    # 4 n-chunks
MT = N // 128     # 16 m-tiles
CK = C // 128     # 8 contraction chunks
SCALE = HD ** -0.5
NCORES = 8
PVLAG = 4         # m-tiles of lead the exp/mask pipeline keeps over PV

F32 = mybir.dt.float32
BF16 = mybir.dt.bfloat16


def _ensure_ntff_hook():
    """bass_utils' trace path imports antenv.axon_hooks, which this image
    lacks; inject it and register the ctypes-based NTFF profile hook."""
    if "antenv.axon_hooks" in sys.modules:
        return
    mod = types.ModuleType("antenv.axon_hooks")
    _hook = [None]
    mod.set_axon_ntff_profile_hook = lambda h: _hook.__setitem__(0, h)
    mod.get_axon_ntff_profile_hook = lambda: _hook[0]
    sys.modules["antenv.axon_hooks"] = mod
    try:
        from trn_agent_boot.trn_boot import _ntff_profile_via_ctypes

        mod.set_axon_ntff_profile_hook(
            _ntff_profile_via_ctypes("/opt/axon/libaxon_pjrt.so")
        )
    except Exception:
        pass


def build():
    nc = bacc.Bacc("TRN2", target_bir_lowering=False, debug=False,
                   num_devices=NCORES)
    xT = nc.dram_tensor("xT", [C, N], BF16, kind="ExternalInput")
    wqk = nc.dram_tensor("wqkT", [C, DQK], BF16, kind="ExternalInput")
    wv = nc.dram_tensor("wvT", [C, DV], BF16, kind="ExternalInput")
    wp = nc.dram_tensor("wpT", [DV, C], BF16, kind="ExternalInput")
    mk = nc.dram_tensor("maskT", [N, N], BF16, kind="ExternalInput")
    yT = nc.dram_tensor("yT", [C, N], F32, kind="ExternalOutput")

    with tile.TileContext(nc) as tc, ExitStack() as ctx:
        consts = ctx.enter_context(tc.tile_pool(name="consts", bufs=1))
        # every x tile has a unique name -> one persistent slot each
        xin = ctx.enter_context(tc.tile_pool(name="xin", bufs=1))
        ptp = ctx.enter_context(tc.tile_pool(name="ptp", bufs=PVLAG + 2))
        ysb = ctx.enter_context(tc.tile_pool(name="ysb", bufs=3))
        dnp = ctx.enter_context(tc.tile_pool(name="dnp", bufs=1))
        rbp = ctx.enter_context(tc.tile_pool(name="rbp", bufs=2))
        # PSUM: "pss" = 2 rotating 2-bank score tiles (also the batched
        # chunk-0 qk accumulators in the head), "pso" = the held O^T pair
        # (own ring so score rotation never waits on it), "p1" = 2 rotating
        # 1-bank tiles (pass accumulators, proj psy) -> 8 banks.
        pool2 = ctx.enter_context(tc.tile_pool(name="pool2", bufs=2, space="PSUM"))
        pool1 = ctx.enter_context(tc.tile_pool(name="pool1", bufs=2, space="PSUM"))

        # ---- resident inputs
        wqk_sb = consts.tile([128, CK, DQK], BF16)
        nc.sync.dma_start(out=wqk_sb,
                          in_=wqk[:].rearrange("(co ci) d -> ci co d", ci=128))
        wv_sb = consts.tile([128, CK, DV], BF16)
        nc.sync.dma_start(out=wv_sb,
                          in_=wv[:].rearrange("(co ci) d -> ci co d", ci=128))
        qk_sb = [consts.tile([128, N], BF16, name=f"qk_m{m}")
                 for m in range(4)]
        vb_sb = consts.tile([128, MT, NH, 128], BF16)
        nc.vector.memset(vb_sb[:, :, :, HD:HD + 1], 1.0)
        nc.vector.memset(vb_sb[:, :, :, HD + 1:], 0.0)
        ot_sb = [consts.tile([128, 2, NCK], BF16, name=f"ot_n{ncb}")
                 for ncb in range(NCH)]
        mask_sb = [consts.tile([128, N], BF16, name=f"mask_m{mt}")
                   for mt in range(MT)]
        wp_sb = consts.tile([128, 2, C], BF16)

        warm = consts.tile([128, NCK], BF16, name="warm")
        nc.vector.memset(warm[:, 0:NCK], 0.0)
        pwarm = pool1.tile([128, NCK], F32, tag="p1", name="pwarm")
        for i in range(10):
            nc.tensor.matmul(pwarm, lhsT=warm[:, 0:128], rhs=warm,
                             start=True, stop=True)

        # ---- x tiles: all 32 up-front (4 MB bf16)
        xts = {}
        for ncb in range(NCH):
            for c in range(CK):
                xt = xin.tile([128, NCK], BF16, name=f"x{ncb}_{c}")
                nc.sync.dma_start(
                    out=xt[:, 0:NCK // 2],
                    in_=xT[c * 128:(c + 1) * 128,
                           ncb * NCK:ncb * NCK + NCK // 2])
                nc.sync.dma_start(
                    out=xt[:, NCK // 2:],
                    in_=xT[c * 128:(c + 1) * 128,
                           ncb * NCK + NCK // 2:(ncb + 1) * NCK])
                xts[(ncb, c)] = xt
        # mask pieces for the first n-chunk follow the x stream directly
        for mt in range(MT):
            nc.sync.dma_start(out=mask_sb[mt][:, 0:NCK],
                              in_=mk[mt * 128:(mt + 1) * 128, 0:NCK])

        # ---- phase-A pass emitters
        def m_pass(ncb, m):
            def th():
                nsl_ = slice(ncb * NCK, (ncb + 1) * NCK)
                pq = pool1.tile([128, NCK], F32, tag="p1",
                                name=f"pa{ncb}m{m}")
                for c in range(CK):
                    nc.tensor.matmul(
                        pq, lhsT=wqk_sb[:, c, m * 128:(m + 1) * 128],
                        rhs=xts[(ncb, c)], start=(c == 0), stop=(c == CK - 1))
                nc.vector.tensor_copy(out=qk_sb[m][:, nsl_], in_=pq)
            return th

        def v_pass_h(ncb, j, h):
            # half V pass: heads 2h..2h+1 only (the hp=h block's needs)
            def th():
                pvt = pool1.tile([128, NCK], F32, tag="p1",
                                 name=f"pv{ncb}v{j}h{h}")
                for c in range(CK):
                    nc.tensor.matmul(
                        pvt[:, 0:128],
                        lhsT=xts[(ncb, c)][:, j * 128:(j + 1) * 128],
                        rhs=wv_sb[:, c, h * 128:(h + 1) * 128],
                        start=(c == 0), stop=(c == CK - 1))
                nc.vector.tensor_copy(
                    out=vb_sb[:, ncb * 4 + j, 2 * h:2 * h + 2, 0:HD],
                    in_=pvt[:, 0:128].rearrange("p (h d) -> p h d", h=2))
            return th

        def mask_piece(ncb, mt):
            def th():
                nsl_ = slice(ncb * NCK, (ncb + 1) * NCK)
                nc.sync.dma_start(out=mask_sb[mt][:, nsl_],
                                  in_=mk[mt * 128:(mt + 1) * 128, nsl_])
            return th

        def wp_load():
            def th():
                nc.sync.dma_start(
                    out=wp_sb,
                    in_=wp[:].rearrange("(dk ci) e -> ci dk e", ci=128))
            return th

        def proj_et(pncb, et, tail=False):
            def th():
                pnsl = slice(pncb * NCK, (pncb + 1) * NCK)
                psy = pool1.tile([128, NCK], F32, tag="p1", name="psy")
                for dk in range(2):
                    nc.tensor.matmul(
                        psy,
                        lhsT=wp_sb[:, dk, et * 128:(et + 1) * 128],
                        rhs=ot_sb[pncb][:, dk, :],
                        start=(dk == 0), stop=(dk == 1))
                yt = ysb.tile([128, NCK], F32)
                # tail: ACT is idle, alternate engines so copies parallelize
                if (tail and et % 2 == 1) or (not tail and et in (3, 7)):
                    nc.scalar.copy(out=yt, in_=psy)
                else:
                    nc.vector.tensor_copy(out=yt, in_=psy)
                nc.sync.dma_start(out=yT[et * 128:(et + 1) * 128, pnsl],
                                  in_=yt)
            return th

        # ---- attention block: 16 mt iterations + weave points
        def block(hp, ncb, weave):
            mq = hp           # qk_sb m-tile holding this pair's Q rows
            mkt = 2 + hp      # qk_sb m-tile holding this pair's K rows
            nsl = slice(ncb * NCK, (ncb + 1) * NCK)
            pso = pool2.tile([128, 2, NCK], F32, tag="pso", bufs=1,
                             name="pso")
            pts = {}
            for mt in range(MT):
                for th in weave.get(mt, ()):
                    th()
                pss = pool2.tile([128, 2, NCK], F32, tag="pss", name="pss")
                for par in range(2):
                    po = par * 64
                    nc.tensor.matmul(
                        pss[:, par, :],
                        lhsT=qk_sb[mkt][po:po + 64,
                                        mt * 128:(mt + 1) * 128],
                        rhs=qk_sb[mq][po:po + 64, nsl],
                        start=True, stop=True)
                pt = ptp.tile([128, 2, NCK], BF16)
                nc.scalar.activation(
                    out=pt, in_=pss,
                    func=mybir.ActivationFunctionType.Exp, scale=SCALE)
                for par in range(2):
                    nc.vector.tensor_mul(out=pt[:, par, :],
                                         in0=pt[:, par, :],
                                         in1=mask_sb[mt][:, nsl])
                pts[mt] = pt
                if mt >= PVLAG:
                    ml = mt - PVLAG
                    for par in range(2):
                        nc.tensor.matmul(
                            pso[:, par, :],
                            lhsT=vb_sb[:, ml, 2 * hp + par, :],
                            rhs=pts[ml][:, par, :],
                            start=(ml == 0), stop=False)
                    del pts[ml]
            for ml in range(MT - PVLAG, MT):
                for par in range(2):
                    nc.tensor.matmul(
                        pso[:, par, :], lhsT=vb_sb[:, ml, 2 * hp + par, :],
                        rhs=pts[ml][:, par, :],
                        start=(ml == 0), stop=(ml == MT - 1))
                del pts[ml]
            # normalize: row 64 of each pso half is the softmax denominator
            den = dnp.tile([1, 2, NCK], F32, tag="den")
            nc.vector.tensor_copy(out=den, in_=pso[HD:HD + 1, :, :])
            rec = dnp.tile([1, 2, NCK], F32, tag="rec")
            nc.vector.reciprocal_approx_fast(out=rec, in_=den)
            rb = rbp.tile([64, 2, NCK], F32)
            nc.gpsimd.partition_broadcast(rb, rec)
            for par in range(2):
                po = par * 64
                nc.vector.tensor_mul(out=ot_sb[ncb][po:po + 64, mq, :],
                                     in0=pso[0:HD, par, :],
                                     in1=rb[:, par, :])

        # ---- head: all four chunk-0 qk m-tiles, batched through the pss
        # ring (phase-A style: two 2-bank accumulators, 4 matmuls per c)
        pa01 = pool2.tile([128, 2, NCK], F32, tag="pss", name="pss")
        pa23 = pool2.tile([128, 2, NCK], F32, tag="pss", name="pss")
        for c in range(CK):
            for m in range(4):
                pa = (pa01, pa23)[m // 2]
                nc.tensor.matmul(
                    pa[:, m % 2, :],
                    lhsT=wqk_sb[:, c, m * 128:(m + 1) * 128],
                    rhs=xts[(0, c)], start=(c == 0), stop=(c == CK - 1))
        for m in range(4):
            pa = (pa01, pa23)[m // 2]
            nc.scalar.copy(out=qk_sb[m][:, 0:NCK], in_=pa[:, m % 2, :])

        # ---- the pipelined stream
        W = {}
        W[(0, 0)] = {
            1: [m_pass(1, 2), v_pass_h(0, 0, 0)],
            2: [v_pass_h(0, 1, 0), v_pass_h(0, 2, 0)],
            3: [v_pass_h(0, 3, 0), v_pass_h(1, 0, 0)],
            4: [v_pass_h(1, 1, 0)],
            5: [m_pass(2, 2), v_pass_h(1, 2, 0)],
            6: [v_pass_h(1, 3, 0), v_pass_h(2, 0, 0)],
            7: [v_pass_h(2, 1, 0)],
            8: [v_pass_h(2, 2, 0)],
            9: [m_pass(3, 2), v_pass_h(2, 3, 0)],
            10: [v_pass_h(3, 0, 0)],
            11: [v_pass_h(3, 1, 0)],
            12: [v_pass_h(3, 2, 0)],
            13: [v_pass_h(3, 3, 0)],
        }
        W[(1, 0)] = {
            0: [wp_load()],
            1: [m_pass(1, 3), v_pass_h(0, 0, 1)],
            2: [v_pass_h(0, 1, 1), v_pass_h(0, 2, 1)],
            3: [v_pass_h(0, 3, 1), v_pass_h(1, 0, 1)],
            4: [v_pass_h(1, 1, 1)],
            5: [m_pass(2, 3), v_pass_h(1, 2, 1)],
            6: [v_pass_h(1, 3, 1), v_pass_h(2, 0, 1)],
            7: [v_pass_h(2, 1, 1)],
            8: [v_pass_h(2, 2, 1)],
            9: [m_pass(3, 3), v_pass_h(2, 3, 1)],
            10: [v_pass_h(3, 0, 1)],
            11: [v_pass_h(3, 1, 1)],
            12: [v_pass_h(3, 2, 1)],
            13: [v_pass_h(3, 3, 1), m_pass(1, 0)],
            14: [m_pass(1, 1)],
        }
        for mt in range(MT):
            W[(1, 0)].setdefault(mt, []).append(mask_piece(1, mt))
        W[(0, 1)] = {13: [m_pass(2, 0)], 14: [m_pass(2, 1)]}
        for mt in range(MT):
            W[(0, 1)].setdefault(mt, []).append(mask_piece(2, mt))
        W[(1, 1)] = {}
        for mt in range(MT):
            W[(1, 1)].setdefault(mt, []).append(mask_piece(3, mt))
        W[(0, 2)] = {13: [m_pass(3, 0)], 14: [m_pass(3, 1)]}
        W[(1, 2)] = {}
        W[(0, 3)] = {}
        W[(1, 3)] = {}
        # output projection for chunk ncb woven into block (0, ncb+1),
        # starting at mt=5 so the PE never queues behind the previous
        # block's normalize chain
        for ncb in range(1, NCH):
            for et in range(8):
                W[(0, ncb)].setdefault(5 + et, []).append(
                    proj_et(ncb - 1, et))

        for ncb in range(NCH):
            for hp in range(2):
                block(hp, ncb, W[(hp, ncb)])
        # tail: output projection for the last n-chunk
        for et in range(8):
            proj_et(NCH - 1, et, tail=True)()


    nc.compile()
    return nc


_NC = None


def _get_nc():
    global _NC
    if _NC is None:
        _NC = build()
    return _NC


def make_in_maps(x, mask, W_qkv, W_proj):
    bf = ml_dtypes.bfloat16
    x = np.asarray(x, dtype=np.float32)
    mask = np.asarray(mask)
    W_qkv = np.asarray(W_qkv, dtype=np.float32)
    W_proj = np.asarray(W_proj, dtype=np.float32)
    in_maps = []
    for cid in range(NCORES):
        b, hg = divmod(cid, 4)
        rs = slice(hg * 256, (hg + 1) * 256)
        wq = W_qkv[0 * C:1 * C][rs]          # [256, 1024]
        wk = W_qkv[1 * C:2 * C][rs]
        wvs = W_qkv[2 * C:3 * C][rs]
        in_maps.append({
            "xT": np.ascontiguousarray(x[b].T).astype(bf),
            "wqkT": np.ascontiguousarray(
                np.concatenate([wq, wk], axis=0).T).astype(bf),
            "wvT": np.ascontiguousarray(wvs.T).astype(bf),
            "wpT": np.ascontiguousarray(W_proj[:, rs].T).astype(bf),
            "maskT": np.ascontiguousarray(mask[b, 0].T).astype(np.float32)
                       .astype(bf),
        })
    return in_maps


LAST_EXEC_NS = None
LAST_MEAN_EXEC_NS = None


def kernel(x, mask, W_qkv, W_proj, b_proj):
    global LAST_EXEC_NS, LAST_MEAN_EXEC_NS
    trace = bool(int(os.environ.get("TRNK_TRACE", "0")))
    if trace:
        _ensure_ntff_hook()
    nc = _get_nc()
    in_maps = make_in_maps(x, mask, W_qkv, W_proj)
    res = run_bass_kernel_spmd(nc, in_maps, list(range(NCORES)), trace=trace)
    LAST_EXEC_NS = res.exec_time_ns
    LAST_MEAN_EXEC_NS = res.mean_exec_time_ns
    y = np.zeros((2, N, C), dtype=np.float32)
    for cid in range(NCORES):
        b = cid // 4
        y[b] += np.asarray(res.results[cid]["yT"], dtype=np.float32).T
    y += np.asarray(b_proj, dtype=np.float32)[None, None, :]
    return y
